# revision 1
# baseline (speedup 1.0000x reference)
"""GATv2 layer (N=50000, D=128, H=4, E=600000) on 8 trn2 NeuronCores.

Strategy (per sharding hint): destination nodes sharded 6250/core; edges
partitioned by destination; xl = h@W_l table replicated (computed on every
core, written to a DRAM table); per-edge work uses indirect-DMA gathers of
xl[src] and xr[dst] rows; segment softmax + scatter-add are done per
128-destination-node window with a selection-matrix matmul that accumulates
both the alpha-weighted feature sum and the softmax denominator in PSUM.
BatchNorm stats are AllReduced across cores.

Softmax is computed without the segment-max subtraction (scores are O(+-10)
in fp32, exp cannot overflow) and normalization is folded to a per-node
divide after aggregation; bias_out is dropped entirely (BatchNorm's mean
subtraction absorbs it exactly).
"""

import math
import numpy as np

import concourse.bass as bass
import concourse.bacc as bacc
import concourse.mybir as mybir
import concourse.tile as tile
from concourse.masks import make_identity
from concourse.bass_utils import run_bass_kernel_spmd

P = 128
F32 = mybir.dt.float32
I32 = mybir.dt.int32

NEG_SLOPE = 0.2
BN_EPS = 1e-5


class Cfg:
    def __init__(self, N, D, H, E, n_cores, K, sup=4):
        assert D == 128
        self.N, self.D, self.H, self.E = N, D, H, E
        self.C = D // H
        self.n_cores = n_cores
        assert N % n_cores == 0
        self.NPC = N // n_cores               # nodes per core
        self.W = math.ceil(self.NPC / P)      # dst windows per core
        self.LASTR = self.NPC - P * (self.W - 1)   # valid rows in last window
        self.NT = math.ceil(N / P)            # h tiles for xl table
        self.K = K                            # edge chunks (of 128) per window
        self.SUP = sup                        # chunks fused per superchunk
        assert K % sup == 0


def build_kernel(nc: bass.Bass, cfg: Cfg, gather_multi: bool = False, debug: bool = False):
    N, H, W, K, SUP = cfg.N, cfg.H, cfg.W, cfg.K, cfg.SUP
    NT, LASTR = cfg.NT, cfg.LASTR
    DEN = P + H   # 132: aggregated features + per-head denominator columns

    # ---- I/O ----
    hfull = nc.declare_dram_parameter("hfull", [NT * P, P], F32, isOutput=False)
    hloc = nc.declare_dram_parameter("hloc", [W * P, P], F32, isOutput=False)
    constsA = nc.declare_dram_parameter("constsA", [P, 259], F32,
                                        isOutput=False)
    constsB = nc.declare_dram_parameter("constsB", [64, P], F32,
                                        isOutput=False)
    srcidx = nc.declare_dram_parameter("srcidx", [W, P, K], I32, isOutput=False)
    dstloc = nc.declare_dram_parameter("dstloc", [W, P, K], I32, isOutput=False)
    dstrel = nc.declare_dram_parameter("dstrel", [W, P, K], F32, isOutput=False)
    out = nc.declare_dram_parameter("out", [cfg.NPC, P], F32, isOutput=True)
    dbg = None
    if debug:
        dbg = nc.declare_dram_parameter("dbg", [cfg.W * P, P], F32,
                                        isOutput=True)
        dbg2 = nc.declare_dram_parameter("dbg2", [cfg.W * P, H], F32,
                                         isOutput=True)

    # ---- internal DRAM ----
    xl_tab = nc.dram_tensor("xl_tab", [NT * P, P], F32)
    xr_tab = nc.dram_tensor("xr_tab", [W * P, P], F32)
    st_in = nc.dram_tensor("st_in", [P, 2], F32)
    st_out = nc.dram_tensor("st_out", [P, 2], F32, addr_space="Shared")

    with tile.TileContext(nc) as tc:
        import contextlib
        with contextlib.ExitStack() as ctx:
            cst = ctx.enter_context(tc.tile_pool(name="cst", bufs=1))
            sb = ctx.enter_context(tc.tile_pool(name="sb", bufs=3))
            ps = ctx.enter_context(tc.tile_pool(name="ps", bufs=4, space="PSUM"))
            psw = ctx.enter_context(tc.tile_pool(name="psw", bufs=2, space="PSUM"))
            ps1 = ctx.enter_context(tc.tile_pool(name="ps1", bufs=1, space="PSUM"))

            # ================= constants (2 coalesced DMAs) ==============
            csA = cst.tile([P, 259], F32, tag="csA")
            nc.sync.dma_start(out=csA[:], in_=constsA[:])
            csB = cst.tile([64, P], F32, tag="csB")
            nc.sync.dma_start(out=csB[:], in_=constsB[:])
            Wl_sb = csA[:, 0:P]
            Wr_sb = csA[:, P:2 * P]
            att_col = csA[:, 256:257]
            gam_col = csA[:, 257:258]
            bet_col = csA[:, 258:259]
            bl_row = csB[0:1, :]
            br_row = csB[32:33, :]

            ones_row = cst.tile([64, P], F32, tag="ones_r")
            nc.gpsimd.memset(ones_row[:], 1.0)
            ones_col = cst.tile([P, 1], F32, tag="ones_c")
            nc.gpsimd.memset(ones_col[:], 1.0)
            ident = cst.tile([P, P], F32, tag="ident")
            make_identity(nc, ident[:])
            eps_col = cst.tile([P, 1], F32, tag="epsc")
            nc.gpsimd.memset(eps_col[:], BN_EPS)
            ones_msk = cst.tile([P, 1], F32, tag="ones_m")
            if LASTR < P:
                pidx = cst.tile([P, 1], I32, tag="pidx")
                nc.gpsimd.iota(pidx[:], pattern=[[0, 1]], channel_multiplier=1)
                nc.vector.tensor_scalar(out=ones_msk[:], in0=pidx[:],
                                        scalar1=LASTR, scalar2=None,
                                        op0=mybir.AluOpType.is_lt)
            else:
                nc.gpsimd.memset(ones_msk[:], 1.0)

            iota_i = cst.tile([P, SUP * P], I32, tag="iota_i")
            nc.gpsimd.iota(iota_i[:], pattern=[[0, SUP], [1, P]],
                           channel_multiplier=0)
            iota_f = cst.tile([P, SUP * P], F32, tag="iota_f")
            nc.vector.tensor_copy(iota_f[:], iota_i[:])

            # att replicated to all partitions: transpose of broadcast column
            att_ps = ps.tile([P, P], F32, tag="p1")
            nc.tensor.transpose(att_ps[:], att_col.to_broadcast([P, P]),
                                ident[:])
            att_rep = cst.tile([P, SUP, P], F32, tag="attrep")
            nc.scalar.copy(
                att_rep[:],
                att_ps[:, None, :].to_broadcast([P, SUP, P]))

            # ================= phase 1: xl table (all nodes) =================
            for t in range(NT):
                ht = sb.tile([P, P], F32, tag="h1")
                nc.sync.dma_start(out=ht[:], in_=hfull[t * P:(t + 1) * P, :])
                hT_ps = ps.tile([P, P], F32, tag="p1")
                nc.tensor.transpose(hT_ps[:], ht[:], ident[:])
                hT = sb.tile([P, P], F32, tag="hT")
                nc.scalar.copy(hT[:], hT_ps[:])
                p1 = ps.tile([P, P], F32, tag="p1")
                nc.tensor.matmul(p1[:], lhsT=hT[:], rhs=Wl_sb,
                                 start=True, stop=False)
                nc.tensor.matmul(p1[:], lhsT=ones_row[0:1, :], rhs=bl_row,
                                 start=False, stop=True)
                xt = sb.tile([P, P], F32, tag="xl1")
                nc.scalar.copy(xt[:], p1[:])
                nc.sync.dma_start(out=xl_tab[t * P:(t + 1) * P, :], in_=xt[:])

            # ================= phase 1b: xr table (local nodes) ==============
            for w in range(W):
                ht = sb.tile([P, P], F32, tag="h1")
                nc.sync.dma_start(out=ht[:], in_=hloc[w * P:(w + 1) * P, :])
                hT_ps = ps.tile([P, P], F32, tag="p1")
                nc.tensor.transpose(hT_ps[:], ht[:], ident[:])
                hT = sb.tile([P, P], F32, tag="hT")
                nc.scalar.copy(hT[:], hT_ps[:])
                p1 = ps.tile([P, P], F32, tag="p1")
                nc.tensor.matmul(p1[:], lhsT=hT[:], rhs=Wr_sb,
                                 start=True, stop=False)
                nc.tensor.matmul(p1[:], lhsT=ones_row[32:33, :], rhs=br_row,
                                 start=False, stop=True)
                xt = sb.tile([P, P], F32, tag="xl1")
                nc.scalar.copy(xt[:], p1[:])
                nc.sync.dma_start(out=xr_tab[w * P:(w + 1) * P, :], in_=xt[:])

            tc.strict_bb_all_engine_barrier()

            # ================= phase 2: edges =================
            stats_ps = ps1.tile([P, 2], F32, tag="stats")
            outpre = []
            for w in range(W):
                op_w = cst.tile([P, P], F32, tag=f"op{w}")
                outpre.append(op_w)

            NSC = K // SUP
            for w in range(W):
                srg = sb.tile([P, K], I32, tag="srg")
                nc.sync.dma_start(out=srg[:], in_=srcidx[w])
                drg = sb.tile([P, K], I32, tag="drg")
                nc.sync.dma_start(out=drg[:], in_=dstloc[w])
                rrg = sb.tile([P, K], F32, tag="rrg")
                nc.sync.dma_start(out=rrg[:], in_=dstrel[w])

                wps = psw.tile([P, DEN], F32, tag="wps")
                for s in range(NSC):
                    G = sb.tile([P, SUP, P], F32, tag="G")
                    if gather_multi:
                        nc.gpsimd.indirect_dma_start(
                            out=G[:], out_offset=None, in_=xl_tab[:],
                            in_offset=bass.IndirectOffsetOnAxis(
                                ap=srg[:, s * SUP:(s + 1) * SUP], axis=0))
                        nc.gpsimd.indirect_dma_start(
                            out=G[:], out_offset=None, in_=xr_tab[:],
                            in_offset=bass.IndirectOffsetOnAxis(
                                ap=drg[:, s * SUP:(s + 1) * SUP], axis=0),
                            compute_op=mybir.AluOpType.add)
                    else:
                        for j in range(SUP):
                            c0 = s * SUP + j
                            nc.gpsimd.indirect_dma_start(
                                out=G[:, j, :], out_offset=None, in_=xl_tab[:],
                                in_offset=bass.IndirectOffsetOnAxis(
                                    ap=srg[:, c0:c0 + 1], axis=0))
                        for j in range(SUP):
                            c0 = s * SUP + j
                            nc.gpsimd.indirect_dma_start(
                                out=G[:, j, :], out_offset=None, in_=xr_tab[:],
                                in_offset=bass.IndirectOffsetOnAxis(
                                    ap=drg[:, c0:c0 + 1], axis=0),
                                compute_op=mybir.AluOpType.add)

                    # z = LeakyReLU(y) = 0.6*y + 0.4*|y| for slope 0.2
                    ab = sb.tile([P, SUP, P], F32, tag="ab")
                    nc.scalar.activation(ab[:], G[:],
                                         mybir.ActivationFunctionType.Abs,
                                         scale=(1.0 - NEG_SLOPE) / 2.0)
                    z = sb.tile([P, SUP, P], F32, tag="z")
                    nc.vector.scalar_tensor_tensor(
                        out=z[:], in0=G[:], scalar=(1.0 + NEG_SLOPE) / 2.0,
                        in1=ab[:], op0=mybir.AluOpType.mult,
                        op1=mybir.AluOpType.add)
                    zz = sb.tile([P, SUP, P], F32, tag="zz")
                    nc.vector.tensor_mul(zz[:], z[:], att_rep[:])
                    s16 = sb.tile([P, SUP * H], F32, tag="s16")
                    nc.vector.tensor_reduce(
                        out=s16[:].rearrange("p (a h) -> p a h", h=H)[:, :, :, None],
                        in_=zz[:].rearrange("p a (h c) -> p a h c", c=cfg.C),
                        op=mybir.AluOpType.add, axis=mybir.AxisListType.X)
                    rhs = sb.tile([P, SUP, DEN], F32, tag="rhs")
                    # alpha = exp(score), written straight into rhs cols 128:132
                    nc.scalar.activation(
                        rhs[:, :, P:DEN],
                        s16[:].rearrange("p (a h) -> p a h", h=H),
                        mybir.ActivationFunctionType.Exp)
                    nc.vector.tensor_mul(
                        rhs[:, :, 0:P].rearrange("p a (h c) -> p a h c", c=cfg.C),
                        G[:].rearrange("p a (h c) -> p a h c", c=cfg.C),
                        rhs[:, :, P:DEN][:, :, :, None]
                            .to_broadcast([P, SUP, H, cfg.C]))
                    sel = sb.tile([P, SUP, P], F32, tag="sel")
                    nc.vector.tensor_tensor(
                        out=sel[:],
                        in0=rrg[:, s * SUP:(s + 1) * SUP][:, :, None]
                            .to_broadcast([P, SUP, P]),
                        in1=iota_f[:].rearrange("p (a c) -> p a c", c=P),
                        op=mybir.AluOpType.is_equal)
                    for j in range(SUP):
                        nc.tensor.matmul(
                            wps[:], lhsT=sel[:, j, :], rhs=rhs[:, j, :],
                            start=(s == 0 and j == 0),
                            stop=(s == NSC - 1 and j == SUP - 1))

                # normalize window: out_pre = agg_y / max(den, tiny) - xr
                # (aggregation used y = xl[src]+xr[dst]; sum(alpha*xl[src]) =
                #  sum(alpha*y) - xr[d]*sum(alpha), exactly)
                xr_win = sb.tile([P, P], F32, tag="xrw")
                nc.sync.dma_start(out=xr_win[:],
                                  in_=xr_tab[w * P:(w + 1) * P, :])
                dmx = sb.tile([P, H], F32, tag="dmx")
                nc.vector.tensor_scalar_max(dmx[:], wps[:, P:DEN], 1e-30)
                rec = sb.tile([P, H], F32, tag="rec")
                nc.vector.reciprocal(rec[:], dmx[:])
                op_w = outpre[w]
                nc.vector.tensor_mul(
                    op_w[:].rearrange("p (h c) -> p h c", c=cfg.C),
                    wps[:, 0:P].rearrange("p (h c) -> p h c", c=cfg.C),
                    rec[:, :, None].to_broadcast([P, H, cfg.C]))
                nc.vector.tensor_sub(op_w[:], op_w[:], xr_win[:])
                if debug:
                    nc.sync.dma_start(out=dbg[w * P:(w + 1) * P, :],
                                      in_=op_w[:])
                    nc.sync.dma_start(out=dbg2[w * P:(w + 1) * P, :],
                                      in_=dmx[:])
                sq = sb.tile([P, P], F32, tag="sq")
                nc.scalar.square(sq[:], op_w[:])
                stat_ones = ones_msk if w == W - 1 else ones_col
                nc.tensor.matmul(stats_ps[:, 0:1], lhsT=op_w[:],
                                 rhs=stat_ones[:],
                                 start=(w == 0), stop=(w == W - 1))
                nc.tensor.matmul(stats_ps[:, 1:2], lhsT=sq[:],
                                 rhs=stat_ones[:],
                                 start=(w == 0), stop=(w == W - 1))

            # ================= phase 3: BN stats AllReduce =================
            st_sb = sb.tile([P, 2], F32, tag="stsb")
            nc.scalar.copy(st_sb[:], stats_ps[:])
            nc.sync.dma_start(out=st_in[:], in_=st_sb[:])
            tc.strict_bb_all_engine_barrier()
            nc.gpsimd.collective_compute(
                "AllReduce", mybir.AluOpType.add,
                replica_groups=[list(range(cfg.n_cores))],
                ins=[st_in[:]], outs=[st_out[:]])
            tc.strict_bb_all_engine_barrier()
            st_all = sb.tile([P, 2], F32, tag="stall")
            nc.sync.dma_start(out=st_all[:], in_=st_out[:])

            # A = gamma * rsqrt(var+eps); B = beta - A*mu  (y = A*x + B)
            mu_c = sb.tile([P, 1], F32, tag="mu")
            nc.scalar.mul(mu_c[:], st_all[:, 0:1], 1.0 / N)
            ex2 = sb.tile([P, 1], F32, tag="ex2")
            nc.scalar.mul(ex2[:], st_all[:, 1:2], 1.0 / N)
            mu2 = sb.tile([P, 1], F32, tag="mu2")
            nc.scalar.square(mu2[:], mu_c[:])
            var_c = sb.tile([P, 1], F32, tag="var")
            nc.vector.tensor_sub(var_c[:], ex2[:], mu2[:])
            sd = sb.tile([P, 1], F32, tag="sd")
            nc.scalar.activation(sd[:], var_c[:],
                                 mybir.ActivationFunctionType.Sqrt,
                                 bias=eps_col[:])
            rsd = sb.tile([P, 1], F32, tag="rsd")
            nc.vector.reciprocal(rsd[:], sd[:])
            A_c = sb.tile([P, 1], F32, tag="Ac")
            nc.vector.tensor_mul(A_c[:], gam_col, rsd[:])
            Amu = sb.tile([P, 1], F32, tag="Amu")
            nc.vector.tensor_mul(Amu[:], A_c[:], mu_c[:])
            B_c = sb.tile([P, 1], F32, tag="Bc")
            nc.vector.tensor_sub(B_c[:], bet_col, Amu[:])

            A_ps = ps.tile([P, P], F32, tag="p1")
            nc.tensor.transpose(A_ps[:], A_c[:].to_broadcast([P, P]), ident[:])
            A_rep = cst.tile([P, P], F32, tag="Arep")
            nc.scalar.copy(A_rep[:], A_ps[:])
            B_ps = ps.tile([P, P], F32, tag="p1")
            nc.tensor.transpose(B_ps[:], B_c[:].to_broadcast([P, P]), ident[:])
            B_rep = cst.tile([P, P], F32, tag="Brep")
            nc.scalar.copy(B_rep[:], B_ps[:])

            # ================= phase 4: BN apply + relu + residual ==========
            for w in range(W):
                rows = P if w < W - 1 else LASTR
                t1 = sb.tile([P, P], F32, tag="t1")
                nc.vector.tensor_mul(t1[:], outpre[w][:], A_rep[:])
                t2 = sb.tile([P, P], F32, tag="t2")
                nc.vector.tensor_add(t2[:], t1[:], B_rep[:])
                r = sb.tile([P, P], F32, tag="r")
                nc.scalar.activation(r[:], t2[:],
                                     mybir.ActivationFunctionType.Relu)
                hres = sb.tile([P, P], F32, tag="hres")
                nc.sync.dma_start(out=hres[:],
                                  in_=hloc[w * P:(w + 1) * P, :])
                o = sb.tile([P, P], F32, tag="o")
                nc.vector.tensor_add(o[:], r[:], hres[:])
                nc.sync.dma_start(out=out[w * P:w * P + rows, :],
                                  in_=o[:rows, :])
    return nc


def host_prepare(h, edge_index, W_l, W_r, bias_l, bias_r, att,
                 bias_out, gamma, beta, n_cores=8, sup=4):
    """Shard edges by destination into per-core, per-window padded chunk
    arrays. Returns (cfg, in_maps)."""
    N, D = h.shape
    H, C = att.shape
    E = edge_index.shape[1]
    h = np.asarray(h, np.float32)
    ei = np.asarray(edge_index)

    loops = np.arange(N, dtype=np.int64)
    src = np.concatenate([ei[0], loops]).astype(np.int64)
    dst = np.concatenate([ei[1], loops]).astype(np.int64)
    order = np.argsort(dst, kind="stable")
    src_s = src[order].astype(np.int32)
    dst_s = dst[order].astype(np.int32)

    NPC = N // n_cores
    W = math.ceil(NPC / P)
    NT = math.ceil(N / P)

    # core boundaries in the dst-sorted edge list
    bounds = np.searchsorted(dst_s, np.arange(0, N + 1, NPC))

    # per (core, window) edge counts -> K
    K = 0
    per_core = []
    for k in range(n_cores):
        lo, hi = bounds[k], bounds[k + 1]
        s_k = src_s[lo:hi]
        dl_k = dst_s[lo:hi] - k * NPC
        win = dl_k // P
        wb = np.searchsorted(win, np.arange(0, W + 1))
        per_core.append((s_k, dl_k, wb))
        for w in range(W):
            cnt = wb[w + 1] - wb[w]
            K = max(K, math.ceil(cnt / P))
    K = max(sup, math.ceil(K / sup) * sup)

    cfg = Cfg(N=N, D=D, H=H, E=E, n_cores=n_cores, K=K, sup=sup)

    hfull = np.zeros((NT * P, P), np.float32)
    hfull[:N] = h

    constsA = np.zeros((P, 259), np.float32)
    constsA[:, 0:P] = np.asarray(W_l, np.float32)
    constsA[:, P:2 * P] = np.asarray(W_r, np.float32)
    constsA[:, 256] = np.asarray(att, np.float32).reshape(-1)
    constsA[:, 257] = np.asarray(gamma, np.float32)
    constsA[:, 258] = np.asarray(beta, np.float32)
    constsB = np.zeros((64, P), np.float32)
    constsB[0] = np.asarray(bias_l, np.float32)
    constsB[32] = np.asarray(bias_r, np.float32)

    in_maps = []
    for k in range(n_cores):
        s_k, dl_k, wb = per_core[k]
        si = np.zeros((W, P, K), np.int32)
        di = np.zeros((W, P, K), np.int32)
        dr = np.full((W, P, K), 300.0, np.float32)
        for w in range(W):
            lo, hi = wb[w], wb[w + 1]
            cnt = hi - lo
            if cnt == 0:
                continue
            buf_s = np.zeros(K * P, np.int32)
            buf_d = np.zeros(K * P, np.int32)
            buf_r = np.full(K * P, 300.0, np.float32)
            buf_s[:cnt] = s_k[lo:hi]
            buf_d[:cnt] = dl_k[lo:hi]
            buf_r[:cnt] = (dl_k[lo:hi] - w * P).astype(np.float32)
            si[w] = buf_s.reshape(K, P).T
            di[w] = buf_d.reshape(K, P).T
            dr[w] = buf_r.reshape(K, P).T
        hloc = np.zeros((W * P, P), np.float32)
        hloc[:NPC] = h[k * NPC:(k + 1) * NPC]
        in_maps.append({
            "hfull": hfull, "hloc": hloc,
            "constsA": constsA, "constsB": constsB,
            "srcidx": si, "dstloc": di, "dstrel": dr,
        })
    return cfg, in_maps


def kernel(h, edge_index, W_l, W_r, bias_l, bias_r, att,
           bias_out, gamma, beta):
    n_cores = 8
    cfg, in_maps = host_prepare(h, edge_index, W_l, W_r, bias_l, bias_r,
                                att, bias_out, gamma, beta, n_cores=n_cores)
    nc = bacc.Bacc()
    build_kernel(nc, cfg)
    nc.compile()
    res = run_bass_kernel_spmd(nc, in_maps, core_ids=list(range(n_cores)))
    outs = [res.results[k]["out"] for k in range(n_cores)]
    return np.concatenate(outs, axis=0).astype(np.float32)



# revision 10
# speedup vs baseline: 2.8829x; 2.8829x over previous
"""GATv2 layer (N=50000, D=128, H=4, E=600000) on 8 trn2 NeuronCores.

Layout: one destination node per SBUF partition row. Nodes are globally
sorted by in-degree, striped across the 8 cores (rank % 8), and packed
into 49 windows of 128 nodes per core; window w pads every node's edge
list to S[w] slots (S[w] = max degree in that window across cores), so
high-degree nodes share windows and padding stays small.

Per window: two dma_gather instructions pull xl[src] rows (bf16, 256B
elements) for all 128*S[w] edge slots — the xl table is split at row
32768 because dma_gather indices are signed int16; slots whose row
lives in the other half gather a dedicated all-zero row, so the halves
just add (no select). xr[dst] is the node's own row broadcast along the
slot axis (no second gather, no selection matmuls). Segment "softmax"
and the alpha-weighted aggregation are row-local vector reduces.

NOTE: this environment's jax.ops.segment_max computes a segment SUM;
the reference subtracts that (not the max) before exp and divides by
(den + 1e-16). We reproduce both quirks exactly — they change the
output materially (some heads collapse to ~0 when den << 1e-16).

xl table rows are stored chunk-permuted (pi) so the phase-1 table write
is one contiguous 4KB-per-partition DMA per 16-tile chunk; the host
remaps gather indices accordingly and un-permutes the output.
"""

import math
import numpy as np
import ml_dtypes

import concourse.bass as bass
import concourse.bacc as bacc
import concourse.mybir as mybir
import concourse.tile as tile
from concourse.masks import make_identity
from concourse.bass_utils import run_bass_kernel_spmd

P = 128
F32 = mybir.dt.float32
BF16 = mybir.dt.bfloat16
I32 = mybir.dt.int32
I16 = mybir.dt.int16
BFNP = ml_dtypes.bfloat16

NEG_SLOPE = 0.2
BN_EPS = 1e-5
HALF = 32768          # dma_gather signed-int16 index limit


class Cfg:
    def __init__(self, N, D, H, n_cores, S):
        assert D == P
        self.N, self.D, self.H = N, D, H
        self.C = D // H
        self.n_cores = n_cores
        self.NPC = N // n_cores              # nodes per core
        self.W = math.ceil(self.NPC / P)     # windows per core
        self.NROWS = self.W * P
        self.LASTR = self.NPC - P * (self.W - 1)
        self.NT = math.ceil(N / P)           # xl table tiles
        self.TAB = self.NT * P
        # [zeros 128] [table TAB] [zeros 128]
        self.TABX = self.TAB + 2 * P
        self.BZERO = self.TAB + P - HALF     # back zero row, rel to HALF
        self.CH = 16                         # tiles per xl-table write chunk
        self.NCH = math.ceil(self.NT / self.CH)
        self.S = [int(s) for s in S]         # slots per window
        offs = np.concatenate([[0], np.cumsum(self.S)])
        self.offs = [int(o) for o in offs]
        self.SS = int(offs[-1])
        self.Smax = int(max(self.S))
        self.J = 7                           # windows per output write group
        self.NG = self.W // self.J
        assert self.W == self.J * self.NG


def build_kernel(nc: bass.Bass, cfg: Cfg, no_gather=False, single_q=False):
    N, H, C, W = cfg.N, cfg.H, cfg.C, cfg.W
    NT, TAB, TABX, CH, NCH = cfg.NT, cfg.TAB, cfg.TABX, cfg.CH, cfg.NCH
    SS, Smax, LASTR = cfg.SS, cfg.Smax, cfg.LASTR
    J, NG = cfg.J, cfg.NG

    # ---- I/O ----
    hfullT = nc.declare_dram_parameter("hfullT", [P, TAB], BF16, isOutput=False)
    hlocT = nc.declare_dram_parameter("hlocT", [P, cfg.NROWS], BF16,
                                      isOutput=False)
    hres2 = nc.declare_dram_parameter("hres2", [cfg.NROWS, P], F32,
                                      isOutput=False)
    constsW = nc.declare_dram_parameter("constsW", [P, 2 * P], BF16,
                                        isOutput=False)
    bias2 = nc.declare_dram_parameter("bias2", [64, P], BF16, isOutput=False)
    constsF = nc.declare_dram_parameter("constsF", [P, 3], F32, isOutput=False)
    idxa = nc.declare_dram_parameter("idxa", [P, SS * 8], I16, isOutput=False)
    idxb = nc.declare_dram_parameter("idxb", [P, SS * 8], I16, isOutput=False)
    maskb = nc.declare_dram_parameter("maskb", [P, SS], F32, isOutput=False)
    out = nc.declare_dram_parameter("out", [cfg.NROWS, P], F32, isOutput=True)

    # ---- internal DRAM ----
    xl_tab = nc.dram_tensor("xl_tab", [TABX, P], BF16)
    st_in = nc.dram_tensor("st_in", [P, 2], F32)
    st_out = nc.dram_tensor("st_out", [P, 2], F32, addr_space="Shared")

    with tile.TileContext(nc) as tc:
        import contextlib
        with contextlib.ExitStack() as ctx:
            cst = ctx.enter_context(tc.tile_pool(name="cst", bufs=1))
            ps = ctx.enter_context(tc.tile_pool(name="ps", bufs=4, space="PSUM"))
            ps1 = ctx.enter_context(tc.tile_pool(name="ps1", bufs=1,
                                                 space="PSUM"))

            # ================= constants =================
            csW = cst.tile([P, 2 * P], BF16, tag="csW")
            nc.sync.dma_start(out=csW[:], in_=constsW[:])
            b2 = cst.tile([64, P], BF16, tag="b2")
            nc.sync.dma_start(out=b2[:], in_=bias2[:])
            csF = cst.tile([P, 3], F32, tag="csF")
            nc.sync.dma_start(out=csF[:], in_=constsF[:])
            ia_sb = cst.tile([P, SS * 8], I16, tag="ia_sb")
            nc.sync.dma_start(out=ia_sb[:], in_=idxa[:])
            ib_sb = cst.tile([P, SS * 8], I16, tag="ib_sb")
            nc.sync.dma_start(out=ib_sb[:], in_=idxb[:])
            msk_sb = cst.tile([P, SS], F32, tag="msk_sb")
            nc.sync.dma_start(out=msk_sb[:], in_=maskb[:])
            msk16 = cst.tile([P, SS], BF16, tag="msk16")
            nc.scalar.copy(msk16[:], msk_sb[:])

            att_col = csF[:, 0:1]
            gam_col = csF[:, 1:2]
            bet_col = csF[:, 2:3]

            ones1_bf = cst.tile([64, P], BF16, tag="ones1bf")
            nc.gpsimd.memset(ones1_bf[:], 1.0)
            ones_col = cst.tile([P, 1], F32, tag="ones_c")
            nc.gpsimd.memset(ones_col[:], 1.0)
            ident = cst.tile([P, P], F32, tag="ident")
            make_identity(nc, ident[:])
            eps_col = cst.tile([P, 1], F32, tag="epsc")
            nc.gpsimd.memset(eps_col[:], BN_EPS)
            ones_msk = cst.tile([P, 1], F32, tag="ones_m")
            if LASTR < P:
                pidx = cst.tile([P, 1], I32, tag="pidx")
                nc.gpsimd.iota(pidx[:], pattern=[[0, 1]], channel_multiplier=1)
                nc.vector.tensor_scalar(out=ones_msk[:], in0=pidx[:],
                                        scalar1=LASTR, scalar2=None,
                                        op0=mybir.AluOpType.is_lt)
            else:
                nc.gpsimd.memset(ones_msk[:], 1.0)

            # att replicated to all partitions (bf16 row)
            att_ps = ps.tile([P, P], F32, tag="p1")
            nc.tensor.transpose(att_ps[:], att_col.to_broadcast([P, P]),
                                ident[:])
            att16 = cst.tile([P, P], BF16, tag="att16")
            nc.scalar.copy(att16[:], att_ps[:])

            # resident per-window data
            xr16 = cst.tile([P, cfg.NROWS], BF16, tag="xr16")
            xr32 = cst.tile([P, cfg.NROWS], F32, tag="xr32")
            outpre = []
            for w in range(W):
                op_w = cst.tile([P, P], F32, tag=f"op{w}")
                outpre.append(op_w)

            # ================= phase 1: xl table (all nodes) =================
            with tc.tile_pool(name="sb1", bufs=3) as sb1:
                # dedicated all-zero rows at both ends of the table
                ztile = sb1.tile([P, P], BF16, tag="ztile")
                nc.gpsimd.memset(ztile[:], 0.0)
                nc.sync.dma_start(
                    out=xl_tab[0:P, :].rearrange("(p x) f -> p (x f)", p=P),
                    in_=ztile[:])
                nc.sync.dma_start(
                    out=xl_tab[TAB + P:TABX, :]
                        .rearrange("(p x) f -> p (x f)", p=P),
                    in_=ztile[:])

                for c in range(NCH):
                    wd = min(CH, NT - c * CH)
                    c0 = c * CH * P
                    hc = sb1.tile([P, CH * P], BF16, tag="hc")
                    nc.sync.dma_start(out=hc[:, :wd * P],
                                      in_=hfullT[:, c0:c0 + wd * P])
                    xlc = sb1.tile([P, CH * P], BF16, tag="xlc")
                    for j in range(wd):
                        p1 = ps.tile([P, P], F32, tag="p1")
                        nc.tensor.matmul(p1[:], lhsT=hc[:, j * P:(j + 1) * P],
                                         rhs=csW[:, 0:P],
                                         start=True, stop=False)
                        nc.tensor.matmul(p1[:], lhsT=ones1_bf[0:1, :],
                                         rhs=b2[0:1, :],
                                         start=False, stop=True)
                        nc.vector.tensor_copy(xlc[:, j * P:(j + 1) * P], p1[:])
                    # rows at P+c0 stored partition-major: row = P+c0+p*wd+j
                    nc.sync.dma_start(
                        out=xl_tab[P + c0:P + c0 + wd * P, :]
                            .rearrange("(p x) f -> p (x f)", p=P),
                        in_=xlc[:, :wd * P])

                # ---- phase 1b: xr for local (permuted) nodes ----
                hl = sb1.tile([P, cfg.NROWS], BF16, tag="hl")
                nc.sync.dma_start(out=hl[:], in_=hlocT[:])
                for w in range(W):
                    p1 = ps.tile([P, P], F32, tag="p1")
                    nc.tensor.matmul(p1[:], lhsT=hl[:, w * P:(w + 1) * P],
                                     rhs=csW[:, P:2 * P],
                                     start=True, stop=False)
                    nc.tensor.matmul(p1[:], lhsT=ones1_bf[32:33, :],
                                     rhs=b2[32:33, :],
                                     start=False, stop=True)
                    nc.scalar.copy(xr16[:, w * P:(w + 1) * P], p1[:])
                    # upcast of the bf16 value => exact cancellation later
                    nc.vector.tensor_copy(xr32[:, w * P:(w + 1) * P],
                                          xr16[:, w * P:(w + 1) * P])

            tc.strict_bb_all_engine_barrier()

            # ================= phase 2: per-window edge processing ==========
            stats_ps = ps1.tile([P, 2], F32, tag="stats")
            with tc.tile_pool(name="sb2", bufs=2) as sb2:
                for w in range(W):
                    S = cfg.S[w]
                    off = cfg.offs[w]
                    NI = S * P
                    wsl = slice(w * P, (w + 1) * P)

                    GA = sb2.tile([P, Smax, P], BF16, tag="GA")
                    GB = sb2.tile([P, Smax, P], BF16, tag="GB")
                    if no_gather:
                        nc.gpsimd.memset(GA[:, :S, :], 0.01)
                        nc.gpsimd.memset(GB[:, :S, :], 0.01)
                    else:
                        nc.gpsimd.dma_gather(
                            out_ap=GA[:, :S, :], in_ap=xl_tab[:HALF, :],
                            idxs_ap=ia_sb[:, off * 8:(off + S) * 8],
                            num_idxs=NI, num_idxs_reg=NI, elem_size=P,
                            queue_num=0, single_packet=False)
                        nc.gpsimd.dma_gather(
                            out_ap=GB[:, :S, :], in_ap=xl_tab[HALF:, :],
                            idxs_ap=ib_sb[:, off * 8:(off + S) * 8],
                            num_idxs=NI, num_idxs_reg=NI, elem_size=P,
                            queue_num=0 if single_q else 1,
                            single_packet=False)

                    # y = xl[src] + xr[dst]  (dst == own row)
                    Y = sb2.tile([P, Smax, P], BF16, tag="Y")
                    nc.vector.tensor_add(Y[:, :S, :], GA[:, :S, :],
                                         GB[:, :S, :])
                    nc.vector.tensor_add(
                        Y[:, :S, :], Y[:, :S, :],
                        xr16[:, wsl][:, None, :].to_broadcast([P, S, P]))

                    # z = LeakyReLU(y) = 0.6*y + 0.4*|y|; zz = z*att (inplace)
                    AB = sb2.tile([P, Smax, P], BF16, tag="AB")
                    nc.scalar.activation(AB[:, :S, :], Y[:, :S, :],
                                         mybir.ActivationFunctionType.Abs,
                                         scale=(1.0 - NEG_SLOPE) / 2.0)
                    nc.vector.scalar_tensor_tensor(
                        out=AB[:, :S, :], in0=Y[:, :S, :],
                        scalar=(1.0 + NEG_SLOPE) / 2.0,
                        in1=AB[:, :S, :], op0=mybir.AluOpType.mult,
                        op1=mybir.AluOpType.add)
                    nc.vector.tensor_mul(
                        AB[:, :S, :], AB[:, :S, :],
                        att16[:, None, :].to_broadcast([P, S, P]))

                    # scores [p, h, s] = sum_c zz
                    s16 = sb2.tile([P, H, Smax], F32, tag="s16")
                    nc.vector.tensor_reduce(
                        out=s16[:, :, :S].rearrange("p h s -> p s h")
                            [:, :, :, None],
                        in_=AB[:, :S, :].rearrange("p s (h c) -> p s h c",
                                                   c=C),
                        op=mybir.AluOpType.add, axis=mybir.AxisListType.X)
                    # zero pad slots (multiplicative mask)
                    sm = sb2.tile([P, H, Smax], F32, tag="sm")
                    nc.vector.tensor_mul(
                        sm[:, :, :S], s16[:, :, :S],
                        msk_sb[:, off:off + S][:, None, :]
                            .to_broadcast([P, H, S]))
                    # segment-SUM subtraction (reference quirk), exp
                    m = sb2.tile([P, H], F32, tag="m")
                    nc.vector.tensor_reduce(
                        out=m[:, :, None], in_=sm[:, :, :S],
                        op=mybir.AluOpType.add, axis=mybir.AxisListType.X)
                    d = sb2.tile([P, H, Smax], F32, tag="d")
                    nc.vector.tensor_sub(
                        d[:, :, :S], sm[:, :, :S],
                        m[:, :, None].to_broadcast([P, H, S]))
                    e16 = sb2.tile([P, H, Smax], BF16, tag="e16")
                    nc.scalar.activation(e16[:, :, :S], d[:, :, :S],
                                         mybir.ActivationFunctionType.Exp)
                    em = sb2.tile([P, H, Smax], BF16, tag="em")
                    nc.vector.tensor_mul(
                        em[:, :, :S], e16[:, :, :S],
                        msk16[:, off:off + S][:, None, :]
                            .to_broadcast([P, H, S]))
                    den = sb2.tile([P, H], F32, tag="den")
                    nc.vector.tensor_reduce(
                        out=den[:, :, None], in_=em[:, :, :S],
                        op=mybir.AluOpType.add, axis=mybir.AxisListType.X)
                    den2 = sb2.tile([P, H], F32, tag="den2")
                    nc.vector.tensor_scalar(out=den2[:], in0=den[:],
                                            scalar1=1e-16, scalar2=None,
                                            op0=mybir.AluOpType.add)
                    rec = sb2.tile([P, H], F32, tag="rec")
                    nc.vector.reciprocal(rec[:], den2[:])
                    fden = sb2.tile([P, H], F32, tag="fden")
                    nc.vector.tensor_mul(fden[:], den[:], rec[:])

                    # weighted aggregation of y, then /(den+eps) and -xr*f
                    WM = sb2.tile([P, Smax, P], BF16, tag="WM")
                    nc.vector.tensor_mul(
                        WM[:, :S, :].rearrange("p s (h c) -> p s h c", c=C),
                        Y[:, :S, :].rearrange("p s (h c) -> p s h c", c=C),
                        em[:, :, :S].rearrange("p h s -> p s h")
                            [:, :, :, None].to_broadcast([P, S, H, C]))
                    op_w = outpre[w]
                    nc.vector.tensor_reduce(
                        out=op_w[:].rearrange("p (h c) -> p h c", c=C)
                            [:, :, :, None],
                        in_=WM[:, :S, :].rearrange("p s (h c) -> p h c s",
                                                   c=C),
                        op=mybir.AluOpType.add, axis=mybir.AxisListType.X)
                    nc.vector.tensor_mul(
                        op_w[:].rearrange("p (h c) -> p h c", c=C),
                        op_w[:].rearrange("p (h c) -> p h c", c=C),
                        rec[:, :, None].to_broadcast([P, H, C]))
                    xrf = sb2.tile([P, P], F32, tag="xrf")
                    nc.vector.tensor_mul(
                        xrf[:].rearrange("p (h c) -> p h c", c=C),
                        xr32[:, wsl].rearrange("p (h c) -> p h c", c=C),
                        fden[:, :, None].to_broadcast([P, H, C]))
                    nc.vector.tensor_sub(op_w[:], op_w[:], xrf[:])

                    # BN stats accumulation
                    sq = sb2.tile([P, P], F32, tag="sq")
                    nc.vector.tensor_mul(sq[:], op_w[:], op_w[:])
                    stat_ones = ones_msk if w == W - 1 else ones_col
                    nc.tensor.matmul(stats_ps[:, 0:1], lhsT=op_w[:],
                                     rhs=stat_ones[:],
                                     start=(w == 0), stop=(w == W - 1))
                    nc.tensor.matmul(stats_ps[:, 1:2], lhsT=sq[:],
                                     rhs=stat_ones[:],
                                     start=(w == 0), stop=(w == W - 1))

            # ================= phase 3: BN stats AllReduce =================
            with tc.tile_pool(name="sb3", bufs=2) as sb:
                st_sb = sb.tile([P, 2], F32, tag="stsb")
                nc.scalar.copy(st_sb[:], stats_ps[:])
                nc.sync.dma_start(out=st_in[:], in_=st_sb[:])
                tc.strict_bb_all_engine_barrier()
                nc.gpsimd.collective_compute(
                    "AllReduce", mybir.AluOpType.add,
                    replica_groups=[list(range(cfg.n_cores))],
                    ins=[st_in[:]], outs=[st_out[:]])
                tc.strict_bb_all_engine_barrier()
                st_all = sb.tile([P, 2], F32, tag="stall")
                nc.sync.dma_start(out=st_all[:], in_=st_out[:])

                # A = gamma * rsqrt(var+eps); B = beta - A*mu  (y = A*x + B)
                mu_c = sb.tile([P, 1], F32, tag="mu")
                nc.scalar.mul(mu_c[:], st_all[:, 0:1], 1.0 / N)
                ex2 = sb.tile([P, 1], F32, tag="ex2")
                nc.scalar.mul(ex2[:], st_all[:, 1:2], 1.0 / N)
                mu2 = sb.tile([P, 1], F32, tag="mu2")
                nc.scalar.square(mu2[:], mu_c[:])
                var_c = sb.tile([P, 1], F32, tag="var")
                nc.vector.tensor_sub(var_c[:], ex2[:], mu2[:])
                sd = sb.tile([P, 1], F32, tag="sd")
                nc.scalar.activation(sd[:], var_c[:],
                                     mybir.ActivationFunctionType.Sqrt,
                                     bias=eps_col[:])
                rsd = sb.tile([P, 1], F32, tag="rsd")
                nc.vector.reciprocal(rsd[:], sd[:])
                A_c = sb.tile([P, 1], F32, tag="Ac")
                nc.vector.tensor_mul(A_c[:], gam_col, rsd[:])
                Amu = sb.tile([P, 1], F32, tag="Amu")
                nc.vector.tensor_mul(Amu[:], A_c[:], mu_c[:])
                B_c = sb.tile([P, 1], F32, tag="Bc")
                nc.vector.tensor_sub(B_c[:], bet_col, Amu[:])

                A_ps = ps.tile([P, P], F32, tag="p1")
                nc.tensor.transpose(A_ps[:], A_c[:].to_broadcast([P, P]),
                                    ident[:])
                A_rep = cst.tile([P, P], F32, tag="Arep")
                nc.scalar.copy(A_rep[:], A_ps[:])
                B_ps = ps.tile([P, P], F32, tag="p1")
                nc.tensor.transpose(B_ps[:], B_c[:].to_broadcast([P, P]),
                                    ident[:])
                B_rep = cst.tile([P, P], F32, tag="Brep")
                nc.scalar.copy(B_rep[:], B_ps[:])

                # ============ phase 4: BN apply + relu + residual ==========
                for g in range(NG):
                    hres = sb.tile([P, J, P], F32, tag="hres")
                    nc.sync.dma_start(
                        out=hres[:],
                        in_=hres2[g * J * P:(g + 1) * J * P, :]
                            .rearrange("(p j) f -> p j f", p=P))
                    obuf = sb.tile([P, J, P], F32, tag="obuf")
                    for j in range(J):
                        w = g * J + j
                        t1 = sb.tile([P, P], F32, tag="t1")
                        nc.vector.tensor_mul(t1[:], outpre[w][:], A_rep[:])
                        t2 = sb.tile([P, P], F32, tag="t2")
                        nc.vector.tensor_add(t2[:], t1[:], B_rep[:])
                        r = sb.tile([P, P], F32, tag="r")
                        nc.scalar.activation(
                            r[:], t2[:], mybir.ActivationFunctionType.Relu)
                        nc.vector.tensor_add(obuf[:, j, :], r[:],
                                             hres[:, j, :])
                    nc.sync.dma_start(
                        out=out[g * J * P:(g + 1) * J * P, :]
                            .rearrange("(p j) f -> p j f", p=P),
                        in_=obuf[:])
    return nc


def host_prepare(h, edge_index, W_l, W_r, bias_l, bias_r, att,
                 bias_out, gamma, beta, n_cores=8):
    N, D = h.shape
    H, C = att.shape
    h = np.asarray(h, np.float32)
    ei = np.asarray(edge_index)

    loops = np.arange(N, dtype=np.int64)
    src = np.concatenate([ei[0], loops]).astype(np.int64)
    dst = np.concatenate([ei[1], loops]).astype(np.int64)
    deg = np.bincount(dst, minlength=N).astype(np.int64)

    # global degree-desc ordering, striped over cores: rank r -> core r%8
    order = np.argsort(-deg, kind="stable")
    NPC = N // n_cores
    node_of = order.reshape(NPC, n_cores)        # [pos, core]
    degmat = deg[node_of]                        # [pos, core]

    W = math.ceil(NPC / P)
    S = np.zeros(W, np.int64)
    for w in range(W):
        i0, i1 = w * P, min((w + 1) * P, NPC)
        S[w] = max(1, degmat[i0:i1].max())
    cfg = Cfg(N=N, D=D, H=H, n_cores=n_cores, S=S)

    # xl table row permutation (chunked partition-major storage)
    n = np.arange(cfg.TAB, dtype=np.int64)
    cc = n // (cfg.CH * P)
    jj = (n % (cfg.CH * P)) // P
    pp = n % P
    width = np.minimum(cfg.CH, cfg.NT - cc * cfg.CH)
    pi = cc * cfg.CH * P + pp * width + jj
    # node n -> table row P + pi[n]
    rowof = P + pi

    # edges grouped by dst, sorted by table row within each group
    eorder = np.lexsort((rowof[src], dst))
    row_s = rowof[src[eorder]].astype(np.int64)
    starts = np.zeros(N + 1, np.int64)
    starts[1:] = np.cumsum(deg)

    # shared inputs
    hT = np.zeros((P, cfg.TAB), BFNP)
    hT[:, :N] = h.T.astype(BFNP)
    constsW = np.zeros((P, 2 * P), BFNP)
    constsW[:, 0:P] = np.asarray(W_l, np.float32).astype(BFNP)
    constsW[:, P:2 * P] = np.asarray(W_r, np.float32).astype(BFNP)
    bias2 = np.zeros((64, P), BFNP)
    bias2[0] = np.asarray(bias_l, np.float32).astype(BFNP)
    bias2[32] = np.asarray(bias_r, np.float32).astype(BFNP)
    constsF = np.zeros((P, 3), np.float32)
    constsF[:, 0] = np.asarray(att, np.float32).reshape(-1)
    constsF[:, 1] = np.asarray(gamma, np.float32)
    constsF[:, 2] = np.asarray(beta, np.float32)

    # output/hres row packing: row2(w, p) = (w//J)*J*P + p*J + (w%J)
    wq, wr = np.divmod(np.arange(W), cfg.J)
    r2map = (wq[:, None] * (cfg.J * P) + np.arange(P)[None, :] * cfg.J
             + wr[:, None])                      # [W, P]

    offs = np.asarray(cfg.offs)
    pos = np.arange(NPC)
    w_k = pos // P
    p_k = pos % P
    colstart = offs[w_k]

    in_maps = []
    for k in range(n_cores):
        nodes = node_of[:, k]
        d_k = deg[nodes]
        tot = int(d_k.sum())
        cum = np.zeros(NPC + 1, np.int64)
        cum[1:] = np.cumsum(d_k)
        within = np.arange(tot, dtype=np.int64) - np.repeat(cum[:-1], d_k)
        rows = np.repeat(p_k, d_k)
        cols = np.repeat(colstart, d_k) + within
        eidx = np.repeat(starts[nodes], d_k) + within

        # slot table row values; pads use the dedicated zero rows
        vals = np.full((P, cfg.SS), -1, np.int64)
        vals[rows, cols] = row_s[eidx]
        isb = vals >= HALF
        ispad = vals < 0
        va = np.where(isb | ispad, 0, vals)              # front zero row
        vb = np.where(isb, vals - HALF, cfg.BZERO)       # back zero row
        maskba = np.zeros((P, cfg.SS), np.float32)
        maskba[rows, cols] = 1.0

        # dma_gather index layout: flat i = s*128+p; wrapped in 16
        # partitions, replicated down all 128 partitions
        def mk_idx(vmat):
            blocks = []
            for w in range(cfg.W):
                o, s_w = cfg.offs[w], cfg.S[w]
                flat = vmat[:, o:o + s_w].T.reshape(-1)   # [s*128+p]
                blk = flat.reshape(s_w * 8, 16).T         # [16, s*8]
                blocks.append(np.tile(blk, (8, 1)))
            return np.concatenate(blocks, axis=1).astype(np.int16)

        idxa = mk_idx(va)
        idxb = mk_idx(vb)

        hlocT = np.zeros((P, cfg.NROWS), BFNP)
        hlocT[:, :NPC] = h[nodes].T.astype(BFNP)
        hres2 = np.zeros((cfg.NROWS, P), np.float32)
        hres2[r2map[w_k, p_k]] = h[nodes]

        in_maps.append({
            "hfullT": hT, "hlocT": hlocT, "hres2": hres2,
            "constsW": constsW, "bias2": bias2, "constsF": constsF,
            "idxa": idxa, "idxb": idxb, "maskb": maskba,
        })

    meta = {"node_of": node_of, "r2map": r2map, "w_k": w_k, "p_k": p_k}
    return cfg, in_maps, meta


def assemble_output(results, cfg, meta, n_cores=8):
    N = cfg.N
    rowsel = meta["r2map"][meta["w_k"], meta["p_k"]]   # [NPC]
    out_full = np.zeros((N, cfg.D), np.float32)
    for k in range(n_cores):
        vals = np.asarray(results[k]["out"], np.float32)[rowsel]
        out_full[meta["node_of"][:, k]] = vals
    return out_full


def kernel(h, edge_index, W_l, W_r, bias_l, bias_r, att,
           bias_out, gamma, beta):
    n_cores = 8
    cfg, in_maps, meta = host_prepare(h, edge_index, W_l, W_r, bias_l,
                                      bias_r, att, bias_out, gamma, beta,
                                      n_cores=n_cores)
    nc = bacc.Bacc(num_swdge_queues=2)
    build_kernel(nc, cfg)
    nc.compile()
    res = run_bass_kernel_spmd(nc, in_maps, core_ids=list(range(n_cores)))
    return assemble_output(res.results, cfg, meta, n_cores=n_cores)
